# revision 6
# baseline (speedup 1.0000x reference)
"""Multi-head causal attention block on 8 Trainium2 NeuronCores.

Sharding: tensor-parallel over heads (4 groups of 4 heads) x data-parallel
over batch (2). Core c -> (batch b=c//4, head-group g=c%4). Each core
computes q/k/v projections for its head group, causal attention for its 4
heads, and a partial output projection; the host sums the 4 partials per
batch. All layout transposes are done host-side so the device does none.

Self-contained: hardcodes shapes for the 2x2048x2048, 16-head problem.
"""

import os
from contextlib import ExitStack

import numpy as np

import concourse.bass as bass
import concourse.tile as tile
from concourse import bacc, mybir
from concourse.bass import ds, ts
from concourse.bass_utils import run_bass_kernel_spmd

F32 = mybir.dt.float32
F32R = mybir.dt.float32r
ACTF = mybir.ActivationFunctionType

# Full-problem dims
BATCH = 2
SEQ = 2048
D_MODEL = 2048
NUM_HEADS = 16
HEAD_DIM = 128
N_CORES = 8
N_GROUPS = 4  # head-groups (tensor parallel)
DG = D_MODEL // N_GROUPS  # 512 = 4 heads per group
SCALE = 1.0 / float(np.sqrt(HEAD_DIM))

QB = 512  # q-block width in attention
KT = 128  # k-tile width (partition dim)

USE_F32R = os.environ.get("KERNEL_F32", "0") != "1"
MMDT = F32R if USE_F32R else F32


def _r(ap):
    """View a float32 DRAM AP as the matmul dtype for DMA into MMDT tiles."""
    return ap.bitcast(F32R) if USE_F32R else ap


def _mha_body(ctx, tc, aps, S, D, DGl):
    """Per-core kernel body.

    aps: dict of DRAM APs: xt [D,S], wqt/wkt/wvt [D,DGl], wot [DGl,D],
      bq/bk [128, DGl//128], bv [128, DGl], bo [128, D], masks [4,128,QB],
      out [S,D].
    """
    nc = tc.nc
    n_heads = DGl // HEAD_DIM  # head tiles per group
    n_kd = D // 128  # contraction tiles over d_model
    n_sq = S // QB  # 512-wide seq blocks
    n_sk = S // KT  # 128-wide seq tiles
    n_dg = DGl // 128

    xt, wqt, wkt, wvt, wot = aps["xt"], aps["wqt"], aps["wkt"], aps["wvt"], aps["wot"]
    out = aps["out"]

    # DRAM scratch for projected q^T/k^T [DGl, S] and v [S, DGl]
    dram = ctx.enter_context(tc.tile_pool(name="dram", bufs=1, space="DRAM"))
    qt_d = dram.tile([DGl, S], F32, name="qt_d")
    kt_d = dram.tile([DGl, S], F32, name="kt_d")
    v_d = dram.tile([S, DGl], F32, name="v_d")

    consts = ctx.enter_context(tc.tile_pool(name="consts", bufs=1))
    ones_sb = consts.tile([128, 1], MMDT, name="ones_sb")
    nc.sync.dma_start(ones_sb[:], _r(aps["ones"]))
    masks_sb = consts.tile([128, 4 * QB], F32, name="masks_sb")
    nc.sync.dma_start(
        masks_sb[:].rearrange("p (i f) -> p i f", i=4),
        aps["masks"].rearrange("i p f -> p i f"),
    )
    bq_sb = consts.tile([128, n_dg], F32, name="bq_sb")
    nc.sync.dma_start(bq_sb[:], aps["bq"])
    bk_sb = consts.tile([128, n_dg], F32, name="bk_sb")
    nc.sync.dma_start(bk_sb[:], aps["bk"])
    bv_sb = consts.tile([128, DGl], F32, name="bv_sb")
    nc.sync.dma_start(bv_sb[:], aps["bv"])
    bo_sb = consts.tile([128, D], F32, name="bo_sb")
    nc.sync.dma_start(bo_sb[:], aps["bo"])

    # ---------------- Phase 1: q/k/v projections ----------------
    with (
        tc.tile_pool(name="wqkv", bufs=1) as wpool,
        tc.tile_pool(name="xt_pool", bufs=2) as xpool,
        tc.tile_pool(name="p1_stage", bufs=6) as stage,
        tc.tile_pool(name="p1_psum", bufs=4, space="PSUM") as psum1,
    ):
        # weights resident: w*_sb[p, k*DGl + f] = w*t[k*128+p, f]
        w_sbs = {}
        for wname, wap in (("wq", wqt), ("wk", wkt), ("wv", wvt)):
            w_sb = wpool.tile([128, n_kd * DGl], MMDT, name=f"{wname}_sb")
            nc.sync.dma_start(
                w_sb[:].rearrange("p (k f) -> p k f", k=n_kd),
                _r(wap).rearrange("(k p) f -> p k f", p=128),
            )
            w_sbs[wname] = w_sb

        for ns in range(n_sq):
            # x^T slice: xt_sb[p, k*QB + f] = x^T[k*128+p, ns*QB+f]
            xt_sb = xpool.tile([128, n_kd * QB], MMDT, tag="xt", name="xt_sb")
            nc.sync.dma_start(
                xt_sb[:].rearrange("p (k f) -> p k f", k=n_kd),
                _r(xt[:, ts(ns, QB)]).rearrange("(k p) f -> p k f", p=128),
            )
            # q^T and k^T tiles: out[m hd-dims 128, QB seq]
            for wname, b_sb, dst in (("wq", bq_sb, qt_d), ("wk", bk_sb, kt_d)):
                w_sb = w_sbs[wname]
                for m in range(n_dg):
                    ps = psum1.tile([128, QB], F32, tag="ps", name="ps_p1")
                    for k in range(n_kd):
                        nc.tensor.matmul(
                            ps[:],
                            lhsT=(w_sb[:, ds(k * DGl + m * 128, 128)]),
                            rhs=(xt_sb[:, ts(k, QB)]),
                            start=(k == 0),
                            stop=(k == n_kd - 1),
                        )
                    st = stage.tile([128, QB], F32, tag="qk_st", name="qk_st")
                    nc.scalar.activation(
                        st[:], ps[:], ACTF.Identity, bias=b_sb[:, ds(m, 1)]
                    )
                    nc.sync.dma_start(dst[ts(m, 128), ts(ns, QB)], st[:])
            # v tiles: out[ms seq 128, DGl hd-cols]
            w_sb = w_sbs["wv"]
            for msub in range(QB // 128):
                ps = psum1.tile([128, DGl], F32, tag="ps", name="ps_p1v")
                for k in range(n_kd):
                    nc.tensor.matmul(
                        ps[:],
                        lhsT=(xt_sb[:, ds(k * QB + msub * 128, 128)]),
                        rhs=(w_sb[:, ts(k, DGl)]),
                        start=(k == 0),
                        stop=(k == n_kd - 1),
                    )
                st = stage.tile([128, DGl], F32, tag="v_st", name="v_st")
                nc.vector.tensor_add(st[:], ps[:], bv_sb[:])
                nc.sync.dma_start(v_d[ds(ns * QB + msub * 128, 128), :], st[:])

    # ---------------- Phase 2: causal attention ----------------
    # ctx^T per head stays resident in SBUF for phase 3
    ctx_pool = ctx.enter_context(tc.tile_pool(name="ctx_pool", bufs=1))
    ctx_sbs = [
        ctx_pool.tile([128, S], MMDT, tag=f"ctx{h}", name=f"ctx_sb{h}")
        for h in range(n_heads)
    ]

    with (
        tc.tile_pool(name="kv_pool", bufs=2) as kvpool,
        tc.tile_pool(name="q_pool", bufs=3) as qpool,
        tc.tile_pool(name="exp_pool", bufs=4) as epool,
        tc.tile_pool(name="lrec_pool", bufs=3) as lpool,
        tc.tile_pool(name="bc_pool", bufs=3) as bcpool,
        tc.tile_pool(name="ps_s", bufs=3, space="PSUM") as ps_s_pool,
        tc.tile_pool(name="ps_c", bufs=2, space="PSUM") as ps_c_pool,
        tc.tile_pool(name="ps_l", bufs=2, space="PSUM") as ps_l_pool,
    ):
        for h in range(n_heads):
            kt_sb = kvpool.tile([128, S], MMDT, tag="kt", name="kt_sb")
            nc.sync.dma_start(kt_sb[:], _r(kt_d[ts(h, 128), :]))
            # v_sb[p, t*128 + j] = v[t*128+p, h*128+j]
            v_sb = kvpool.tile([128, n_sk * 128], MMDT, tag="v", name="v_sb")
            nc.sync.dma_start(
                v_sb[:].rearrange("p (t j) -> p t j", t=n_sk),
                _r(v_d[:, ts(h, 128)]).rearrange("(t p) j -> p t j", p=128),
            )
            for qb in range(n_sq):
                q_sb = qpool.tile([128, QB], MMDT, tag="q", name="q_sb")
                nc.sync.dma_start(q_sb[:], _r(qt_d[ts(h, 128), ts(qb, QB)]))
                n_kt = (qb + 1) * (QB // KT)  # causal: only k-tiles <= q
                ps_c = ps_c_pool.tile([128, QB], F32, tag="c", name="ps_c")
                ps_l = ps_l_pool.tile([1, QB], F32, tag="l", name="ps_l")
                diag0 = n_kt - (QB // KT)
                for kt in range(n_kt):
                    ps_sc = ps_s_pool.tile([128, QB], F32, tag="s", name="ps_sc")
                    nc.tensor.matmul(
                        ps_sc[:],
                        lhsT=(kt_sb[:, ts(kt, 128)]),
                        rhs=(q_sb[:]),
                        start=True,
                        stop=True,
                    )
                    if kt >= diag0:
                        off = kt - diag0
                        nc.vector.tensor_add(
                            ps_sc[:], ps_sc[:], masks_sb[:, ts(off, QB)]
                        )
                    ex = epool.tile([128, QB], MMDT, tag="e", name="ex")
                    nc.scalar.activation(ex[:], ps_sc[:], ACTF.Exp, scale=SCALE)
                    nc.tensor.matmul(
                        ps_c[:],
                        lhsT=(v_sb[:, ts(kt, 128)]),
                        rhs=(ex[:]),
                        start=(kt == 0),
                        stop=(kt == n_kt - 1),
                    )
                    nc.tensor.matmul(
                        ps_l[:],
                        lhsT=(ones_sb[:]),
                        rhs=(ex[:]),
                        start=(kt == 0),
                        stop=(kt == n_kt - 1),
                    )
                rec = lpool.tile([1, QB], F32, tag="r", name="rec")
                nc.vector.reciprocal(rec[:], ps_l[:])
                bc = bcpool.tile([128, QB], F32, tag="bc", name="bc")
                nc.gpsimd.partition_broadcast(bc[:], rec[:])
                nc.vector.tensor_mul(
                    ctx_sbs[h][:, ts(qb, QB)], ps_c[:], bc[:]
                )

    # ---------------- Phase 3: output projection ----------------
    with (
        tc.tile_pool(name="wo_pool", bufs=1) as wopool,
        tc.tile_pool(name="o_stage", bufs=4) as ostage,
        tc.tile_pool(name="p3_psum", bufs=4, space="PSUM") as psum3,
    ):
        # wo_sb[p, k*D + f] = wot[k*128+p, f]
        wo_sb = wopool.tile([128, n_dg * D], MMDT, name="wo_sb")
        nc.sync.dma_start(
            wo_sb[:].rearrange("p (k f) -> p k f", k=n_dg),
            _r(wot).rearrange("(k p) f -> p k f", p=128),
        )
        for m in range(n_sk):
            for n in range(D // QB):
                ps = psum3.tile([128, QB], F32, tag="o", name="ps_p3")
                for k in range(n_dg):
                    nc.tensor.matmul(
                        ps[:],
                        lhsT=(ctx_sbs[k][:, ts(m, 128)]),
                        rhs=(wo_sb[:, ds(k * D + n * QB, QB)]),
                        start=(k == 0),
                        stop=(k == n_dg - 1),
                    )
                ot = ostage.tile([128, QB], F32, tag="ot", name="ot")
                nc.vector.tensor_add(ot[:], ps[:], bo_sb[:, ts(n, QB)])
                nc.sync.dma_start(out[ts(m, 128), ts(n, QB)], ot[:])


def build_program(S=SEQ, D=D_MODEL, DGl=DG, enable_asserts=False):
    nc = bacc.Bacc(
        "TRN2",
        target_bir_lowering=False,
        debug=False,
        enable_asserts=enable_asserts,
        num_devices=N_CORES,
    )
    aps = {
        "xt": nc.dram_tensor("xt", [D, S], F32, kind="ExternalInput").ap(),
        "wqt": nc.dram_tensor("wqt", [D, DGl], F32, kind="ExternalInput").ap(),
        "wkt": nc.dram_tensor("wkt", [D, DGl], F32, kind="ExternalInput").ap(),
        "wvt": nc.dram_tensor("wvt", [D, DGl], F32, kind="ExternalInput").ap(),
        "wot": nc.dram_tensor("wot", [DGl, D], F32, kind="ExternalInput").ap(),
        "bq": nc.dram_tensor("bq", [128, DGl // 128], F32, kind="ExternalInput").ap(),
        "bk": nc.dram_tensor("bk", [128, DGl // 128], F32, kind="ExternalInput").ap(),
        "bv": nc.dram_tensor("bv", [128, DGl], F32, kind="ExternalInput").ap(),
        "bo": nc.dram_tensor("bo", [128, D], F32, kind="ExternalInput").ap(),
        "masks": nc.dram_tensor("masks", [4, 128, QB], F32, kind="ExternalInput").ap(),
        "ones": nc.dram_tensor("ones", [128, 1], F32, kind="ExternalInput").ap(),
        "out": nc.dram_tensor("out", [S, D], F32, kind="ExternalOutput").ap(),
    }
    with tile.TileContext(nc) as tc:
        with ExitStack() as ctx:
            _mha_body(ctx, tc, aps, S, D, DGl)
    nc.compile()
    return nc


def make_masks():
    """Additive causal masks: 0 where k<=q, -1e30 where masked."""
    i = np.arange(4)[:, None, None]
    p = np.arange(128)[None, :, None]
    f = np.arange(QB)[None, None, :]
    keep = (i * 128 + p) <= f
    return np.where(keep, 0.0, -1e30).astype(np.float32)


def shard_inputs(x, wq, bq, wk, bk, wv, bv, wo, bo):
    """Build the 8 per-core input maps (host-side layout prep)."""
    masks = make_masks()
    xts = [np.ascontiguousarray(np.asarray(x[b], np.float32).T) for b in range(BATCH)]
    bo_bc = np.ascontiguousarray(
        np.broadcast_to(np.asarray(bo, np.float32), (128, D_MODEL))
    )
    bo_zero = np.zeros((128, D_MODEL), np.float32)
    in_maps = []
    for c in range(N_CORES):
        b, g = divmod(c, N_GROUPS)
        sl = slice(g * DG, (g + 1) * DG)
        in_maps.append(
            {
                "xt": xts[b],
                "wqt": np.ascontiguousarray(np.asarray(wq, np.float32)[sl].T),
                "wkt": np.ascontiguousarray(np.asarray(wk, np.float32)[sl].T),
                "wvt": np.ascontiguousarray(np.asarray(wv, np.float32)[sl].T),
                "wot": np.ascontiguousarray(np.asarray(wo, np.float32)[:, sl].T),
                "bq": np.ascontiguousarray(
                    np.asarray(bq, np.float32)[sl].reshape(-1, 128).T
                ),
                "bk": np.ascontiguousarray(
                    np.asarray(bk, np.float32)[sl].reshape(-1, 128).T
                ),
                "bv": np.ascontiguousarray(
                    np.broadcast_to(np.asarray(bv, np.float32)[sl], (128, DG))
                ),
                "bo": bo_bc if g == 0 else bo_zero,
                "masks": masks,
                "ones": np.ones((128, 1), np.float32),
            }
        )
    return in_maps


_NC_CACHE = {}


def get_program():
    if "nc" not in _NC_CACHE:
        _NC_CACHE["nc"] = build_program()
    return _NC_CACHE["nc"]


def run_sharded(inputs, trace=False):
    nc = get_program()
    in_maps = shard_inputs(**inputs)
    res = run_bass_kernel_spmd(nc, in_maps, list(range(N_CORES)), trace=trace)
    full = np.empty((BATCH, SEQ, D_MODEL), np.float32)
    for b in range(BATCH):
        acc = res.results[b * N_GROUPS]["out"].copy()
        for g in range(1, N_GROUPS):
            acc += res.results[b * N_GROUPS + g]["out"]
        full[b] = acc
    return full, res


def kernel(**inputs):
    out, _ = run_sharded(inputs, trace=False)
    return out


# revision 19
# speedup vs baseline: 1.2091x; 1.2091x over previous
"""Multi-head causal attention block on 8 Trainium2 NeuronCores.

Sharding: tensor-parallel over heads (4 groups of 4 heads) x data-parallel
over batch (2). Core c -> (batch b=c//4, head-group g=c%4). Each core
computes q/k/v projections for its head group, causal attention for its 4
heads, and a partial output projection; the host sums the 4 partials per
batch. All layout transposes are done host-side so the device does none.

Self-contained: hardcodes shapes for the 2x2048x2048, 16-head problem.
"""

import os
from contextlib import ExitStack

import numpy as np

import concourse.bass as bass
import concourse.tile as tile
from concourse import bacc, mybir
from concourse.bass import ds, ts
from concourse.bass_utils import run_bass_kernel_spmd

F32 = mybir.dt.float32
F32R = mybir.dt.float32r
ACTF = mybir.ActivationFunctionType

# Full-problem dims
BATCH = 2
SEQ = 2048
D_MODEL = 2048
NUM_HEADS = 16
HEAD_DIM = 128
N_CORES = 8
N_GROUPS = 4  # head-groups (tensor parallel)
DG = D_MODEL // N_GROUPS  # 512 = 4 heads per group
SCALE = 1.0 / float(np.sqrt(HEAD_DIM))

QB = 512  # q-block width in attention
KT = 128  # k-tile width (partition dim)

USE_F32R = os.environ.get("KERNEL_F32", "0") != "1"
MMDT = F32R if USE_F32R else F32


def _r(ap):
    """View a float32 DRAM AP as the matmul dtype for DMA into MMDT tiles."""
    return ap.bitcast(F32R) if USE_F32R else ap


def _mha_body(ctx, tc, aps, S, D, DGl):
    """Per-core kernel body.

    aps: dict of DRAM APs: xt [D,S], wqt/wkt/wvt [D,DGl], wot [DGl,D],
      bq/bk [128, DGl//128], bv [128, DGl], bo [128, D], masks [4,128,QB],
      ones [128,1], out [S,D].

    k^T and v stay resident in SBUF (written directly by the projection
    drains); only q^T round-trips through DRAM.
    """
    nc = tc.nc
    n_kd = D // 128  # contraction tiles over d_model
    n_sq = S // QB  # 512-wide attention q-blocks
    n_sk = S // KT  # 128-wide seq tiles
    n_dg = DGl // 128  # head tiles per group
    QB1 = 256  # phase-1 seq-slice width
    n_ns = S // QB1

    xt, wqt, wkt, wvt, wot = aps["xt"], aps["wqt"], aps["wkt"], aps["wvt"], aps["wot"]
    out = aps["out"]

    # DRAM scratch for v [S, DGl] (q^T and k^T stay resident in SBUF)
    dram = ctx.enter_context(tc.tile_pool(name="dram", bufs=1, space="DRAM"))
    v_d = dram.tile([S, DGl], F32, name="v_d")

    consts = ctx.enter_context(tc.tile_pool(name="consts", bufs=1))
    # dummy activation first: forces the ACT function-table DMA to queue
    # ahead of the bulk input loads (else every early PSUM drain stalls)
    warm = consts.tile([128, 1], F32, name="act_warm")
    nc.vector.memset(warm[:], 0.0)
    nc.scalar.activation(warm[:], warm[:], ACTF.Identity, bias=warm[:, 0:1])
    ones_sb = consts.tile([128, 1], MMDT, name="ones_sb")
    bq_sb = consts.tile([128, n_dg], F32, name="bq_sb")
    bk_sb = consts.tile([128, n_dg], F32, name="bk_sb")
    bv_sb = consts.tile([128, DGl], F32, name="bv_sb")
    masks_sb = consts.tile([128, 4 * QB], F32, name="masks_sb")

    # resident q^T / k^T per head: [p, s] = q^T/k^T[h*128+p, s]
    kv_pool = ctx.enter_context(tc.tile_pool(name="kv_res", bufs=1))
    kt_res = [
        kv_pool.tile([128, S], MMDT, tag=f"ktr{h}", name=f"kt_res{h}")
        for h in range(n_dg)
    ]
    qt_res = [
        kv_pool.tile([128, S], MMDT, tag=f"qtr{h}", name=f"qt_res{h}")
        for h in range(n_dg)
    ]

    # ---------------- Phase 1: q/k/v projections ----------------
    with (
        tc.tile_pool(name="wqkv", bufs=1) as wpool,
        tc.tile_pool(name="xt_pool", bufs=2) as xpool,
        tc.tile_pool(name="p1_stage", bufs=2) as stage,
        tc.tile_pool(name="p1_psum", bufs=4, space="PSUM") as psum1,
    ):
        # weights resident: w*_sb[p, k*DGl + f] = w*t[k*128+p, f]
        w_sbs = {
            wname: wpool.tile([128, n_kd * DGl], MMDT, name=f"{wname}_sb")
            for wname in ("wq", "wk", "wv")
        }

        def load_w(wname, wap):
            nc.sync.dma_start(
                w_sbs[wname][:].rearrange("p (k f) -> p k f", k=n_kd),
                _r(wap).rearrange("(k p) f -> p k f", p=128),
            )

        def load_w_mblock(wname, wap, m):
            nc.sync.dma_start(
                w_sbs[wname][:].rearrange(
                    "p (k g j) -> p k g j", k=n_kd, j=128
                )[:, :, m, :],
                _r(wap).rearrange("(k p) (g j) -> p k g j", p=128, j=128)[
                    :, :, m, :
                ],
            )

        def load_xt(ns):
            t = xpool.tile([128, n_kd * QB1], MMDT, tag="xt", name="xt_sb")
            nc.sync.dma_start(
                t[:].rearrange("p (k f) -> p k f", k=n_kd),
                _r(xt[:, ts(ns, QB1)]).rearrange("(k p) f -> p k f", p=128),
            )
            return t

        nc.sync.dma_start(ones_sb[:], _r(aps["ones"]))
        nc.sync.dma_start(bq_sb[:], aps["bq"])
        nc.sync.dma_start(bk_sb[:], aps["bk"])
        nc.sync.dma_start(bv_sb[:], aps["bv"])
        # k^T first: PE can start on wk+x0 while wq/wv still stream in
        load_w_mblock("wk", wkt, 0)
        g0 = load_xt(0)
        for m in range(1, n_dg):
            load_w_mblock("wk", wkt, m)
        g1 = load_xt(1)
        load_w("wq", wqt)
        load_w("wv", wvt)
        nc.sync.dma_start(
            masks_sb[:].rearrange("p (i f) -> p i f", i=4),
            aps["masks"].rearrange("i p f -> p i f"),
        )

        def do_proj_t(res, wname, b_sb, ns, xt_sb):
            # q^T/k^T [m hd-dims 128, QB1 seq] drains into resident tiles
            for m in range(n_dg):
                ps = psum1.tile([128, QB1], F32, tag="ps", name="ps_qk")
                for k in range(n_kd):
                    nc.tensor.matmul(
                        ps[:],
                        lhsT=w_sbs[wname][:, ds(k * DGl + m * 128, 128)],
                        rhs=xt_sb[:, ts(k, QB1)],
                        start=(k == 0),
                        stop=(k == n_kd - 1),
                    )
                nc.scalar.activation(
                    res[m][:, ts(ns, QB1)],
                    ps[:],
                    ACTF.Identity,
                    bias=b_sb[:, ds(m, 1)],
                )

        def do_v(ns, xt_sb):
            for msub in range(QB1 // 128):
                ps = psum1.tile([128, DGl], F32, tag="ps", name="ps_v")
                for k in range(n_kd):
                    nc.tensor.matmul(
                        ps[:],
                        lhsT=xt_sb[:, ds(k * QB1 + msub * 128, 128)],
                        rhs=w_sbs["wv"][:, ts(k, DGl)],
                        start=(k == 0),
                        stop=(k == n_kd - 1),
                    )
                st = stage.tile([128, DGl], F32, tag="v_st", name="v_st")
                nc.vector.tensor_add(st[:], ps[:], bv_sb[:])
                nc.sync.dma_start(
                    v_d[ds(ns * QB1 + msub * 128, 128), :], st[:]
                )

        # head group: k^T for slices 0-1 (no DMA drains), then q^T, then v
        for ns, g in ((0, g0), (1, g1)):
            do_proj_t(kt_res, "wk", bk_sb, ns, g)
        for ns, g in ((0, g0), (1, g1)):
            do_proj_t(qt_res, "wq", bq_sb, ns, g)
        for ns, g in ((0, g0), (1, g1)):
            do_v(ns, g)
        nxt = load_xt(2) if n_ns > 2 else None
        for ns in range(2, n_ns):
            xt_sb = nxt
            nxt = load_xt(ns + 1) if ns + 1 < n_ns else None
            do_proj_t(kt_res, "wk", bk_sb, ns, xt_sb)
            do_proj_t(qt_res, "wq", bq_sb, ns, xt_sb)
            do_v(ns, xt_sb)

    # ---------------- Phase 2: causal attention ----------------
    # ctx^T per head stays resident in SBUF for phase 3
    ctx_pool = ctx.enter_context(tc.tile_pool(name="ctx_pool", bufs=1))
    ctx_sbs = [
        ctx_pool.tile([128, S], MMDT, tag=f"ctx{h}", name=f"ctx_sb{h}")
        for h in range(n_dg)
    ]

    # wo stays resident; loaded mid-phase-2 so phase 3 starts hot
    wopool = ctx.enter_context(tc.tile_pool(name="wo_pool", bufs=1))
    wo_sb = wopool.tile([128, n_dg * D], MMDT, name="wo_sb")

    # phase-2/3-only constants live after phase-1 pools are freed
    p2consts = ctx.enter_context(tc.tile_pool(name="p2consts", bufs=1))
    bo_sb = p2consts.tile([128, D], F32, name="bo_sb")
    nc.sync.dma_start(bo_sb[:], aps["bo"])

    with (
        tc.tile_pool(name="v_pool", bufs=2) as vpool,
        tc.tile_pool(name="exp_pool", bufs=8) as epool,
        tc.tile_pool(name="lrec_pool", bufs=3) as lpool,
        tc.tile_pool(name="bc_pool", bufs=3) as bcpool,
        tc.tile_pool(name="ps_s", bufs=3, space="PSUM") as ps_s_pool,
        tc.tile_pool(name="ps_c", bufs=3, space="PSUM") as ps_c_pool,
        tc.tile_pool(name="ps_l", bufs=2, space="PSUM") as ps_l_pool,
    ):
        for h in range(n_dg):
            # v_sb[p, t*128+j] = v[t*128+p, h*128+j]; quarter DMAs so the
            # first q-blocks' PV can start before the whole head lands
            v_sb = vpool.tile([128, n_sk * 128], MMDT, tag="v", name="v_sb")
            nq = max(1, S // 512)
            for vq in range(nq):
                nc.sync.dma_start(
                    v_sb[:, ds(vq * 512, 512)].rearrange(
                        "p (t j) -> p t j", j=128
                    ),
                    _r(v_d[ds(vq * 512, 512), ts(h, 128)]).rearrange(
                        "(t p) j -> p t j", p=128
                    ),
                )
            if h == 1:
                # wo_sb[p, k*D + f] = wot[k*128+p, f] (phase-3 prefetch)
                nc.sync.dma_start(
                    wo_sb[:].rearrange("p (k f) -> p k f", k=n_dg),
                    _r(wot).rearrange("(k p) f -> p k f", p=128),
                )
            for qb in range(n_sq):
                n_kt = (qb + 1) * (QB // KT)  # causal: only k-tiles <= q
                ps_c = ps_c_pool.tile([128, QB], F32, tag="c", name="ps_c")
                ps_l = ps_l_pool.tile([1, QB], F32, tag="l", name="ps_l")
                diag0 = n_kt - (QB // KT)
                for kt in range(n_kt):
                    off = kt - diag0
                    # causal column restriction: diagonal tile off needs
                    # only cols >= off*128; keep moving dim >= 256 for
                    # full-rate f32r (so off=3 starts at 256, masked).
                    sc = 0 if off < 1 else (128 if off == 1 else 256)
                    w = QB - sc
                    ps_sc = ps_s_pool.tile([128, QB], F32, tag="s", name="ps_sc")
                    nc.tensor.matmul(
                        ps_sc[:, ds(sc, w)],
                        lhsT=kt_res[h][:, ts(kt, 128)],
                        rhs=qt_res[h][:, ds(qb * QB + sc, w)],
                        start=True,
                        stop=True,
                    )
                    if off >= 0:
                        nc.vector.tensor_add(
                            ps_sc[:, ds(sc, w)],
                            ps_sc[:, ds(sc, w)],
                            masks_sb[:, ds(off * QB + sc, w)],
                        )
                    ex = epool.tile([128, QB], MMDT, tag="e", name="ex")
                    nc.scalar.activation(
                        ex[:, ds(sc, w)], ps_sc[:, ds(sc, w)], ACTF.Exp, scale=SCALE
                    )
                    nc.tensor.matmul(
                        ps_c[:, ds(sc, w)],
                        lhsT=v_sb[:, ts(kt, 128)],
                        rhs=ex[:, ds(sc, w)],
                        start=(kt == 0),
                        stop=(kt == n_kt - 1),
                        skip_group_check=True,
                    )
                    nc.tensor.matmul(
                        ps_l[:, ds(sc, w)],
                        lhsT=ones_sb[:],
                        rhs=ex[:, ds(sc, w)],
                        start=(kt == 0),
                        stop=(kt == n_kt - 1),
                        skip_group_check=True,
                    )
                rec = lpool.tile([1, QB], F32, tag="r", name="rec")
                nc.vector.reciprocal(rec[:], ps_l[:])
                bc = bcpool.tile([128, QB], F32, tag="bc", name="bc")
                nc.gpsimd.partition_broadcast(bc[:], rec[:])
                nc.vector.tensor_mul(
                    ctx_sbs[h][:, ts(qb, QB)], ps_c[:], bc[:]
                )

    # ---------------- Phase 3: output projection ----------------
    with (
        tc.tile_pool(name="o_stage", bufs=4) as ostage,
        tc.tile_pool(name="p3_psum", bufs=4, space="PSUM") as psum3,
    ):
        for m in range(n_sk):
            for n in range(D // QB):
                ps = psum3.tile([128, QB], F32, tag="o", name="ps_p3")
                for k in range(n_dg):
                    nc.tensor.matmul(
                        ps[:],
                        lhsT=ctx_sbs[k][:, ts(m, 128)],
                        rhs=wo_sb[:, ds(k * D + n * QB, QB)],
                        start=(k == 0),
                        stop=(k == n_dg - 1),
                    )
                ot = ostage.tile([128, QB], F32, tag="ot", name="ot")
                nc.vector.tensor_add(ot[:], ps[:], bo_sb[:, ts(n, QB)])
                nc.sync.dma_start(out[ts(m, 128), ts(n, QB)], ot[:])


def build_program(S=SEQ, D=D_MODEL, DGl=DG, enable_asserts=False):
    nc = bacc.Bacc(
        "TRN2",
        target_bir_lowering=False,
        debug=False,
        enable_asserts=enable_asserts,
        num_devices=N_CORES,
    )
    aps = {
        "xt": nc.dram_tensor("xt", [D, S], F32, kind="ExternalInput").ap(),
        "wqt": nc.dram_tensor("wqt", [D, DGl], F32, kind="ExternalInput").ap(),
        "wkt": nc.dram_tensor("wkt", [D, DGl], F32, kind="ExternalInput").ap(),
        "wvt": nc.dram_tensor("wvt", [D, DGl], F32, kind="ExternalInput").ap(),
        "wot": nc.dram_tensor("wot", [DGl, D], F32, kind="ExternalInput").ap(),
        "bq": nc.dram_tensor("bq", [128, DGl // 128], F32, kind="ExternalInput").ap(),
        "bk": nc.dram_tensor("bk", [128, DGl // 128], F32, kind="ExternalInput").ap(),
        "bv": nc.dram_tensor("bv", [128, DGl], F32, kind="ExternalInput").ap(),
        "bo": nc.dram_tensor("bo", [128, D], F32, kind="ExternalInput").ap(),
        "masks": nc.dram_tensor("masks", [4, 128, QB], F32, kind="ExternalInput").ap(),
        "ones": nc.dram_tensor("ones", [128, 1], F32, kind="ExternalInput").ap(),
        "out": nc.dram_tensor("out", [S, D], F32, kind="ExternalOutput").ap(),
    }
    with tile.TileContext(nc) as tc:
        with ExitStack() as ctx:
            _mha_body(ctx, tc, aps, S, D, DGl)
    nc.compile()
    return nc


def make_masks():
    """Additive causal masks: 0 where k<=q, -1e30 where masked."""
    i = np.arange(4)[:, None, None]
    p = np.arange(128)[None, :, None]
    f = np.arange(QB)[None, None, :]
    keep = (i * 128 + p) <= f
    return np.where(keep, 0.0, -1e30).astype(np.float32)


def shard_inputs(x, wq, bq, wk, bk, wv, bv, wo, bo):
    """Build the 8 per-core input maps (host-side layout prep)."""
    masks = make_masks()
    xts = [np.ascontiguousarray(np.asarray(x[b], np.float32).T) for b in range(BATCH)]
    bo_bc = np.ascontiguousarray(
        np.broadcast_to(np.asarray(bo, np.float32), (128, D_MODEL))
    )
    bo_zero = np.zeros((128, D_MODEL), np.float32)
    in_maps = []
    for c in range(N_CORES):
        b, g = divmod(c, N_GROUPS)
        sl = slice(g * DG, (g + 1) * DG)
        in_maps.append(
            {
                "xt": xts[b],
                "wqt": np.ascontiguousarray(np.asarray(wq, np.float32)[sl].T),
                "wkt": np.ascontiguousarray(np.asarray(wk, np.float32)[sl].T),
                "wvt": np.ascontiguousarray(np.asarray(wv, np.float32)[sl].T),
                "wot": np.ascontiguousarray(np.asarray(wo, np.float32)[:, sl].T),
                "bq": np.ascontiguousarray(
                    np.asarray(bq, np.float32)[sl].reshape(-1, 128).T
                ),
                "bk": np.ascontiguousarray(
                    np.asarray(bk, np.float32)[sl].reshape(-1, 128).T
                ),
                "bv": np.ascontiguousarray(
                    np.broadcast_to(np.asarray(bv, np.float32)[sl], (128, DG))
                ),
                "bo": bo_bc if g == 0 else bo_zero,
                "masks": masks,
                "ones": np.ones((128, 1), np.float32),
            }
        )
    return in_maps


_NC_CACHE = {}


def get_program():
    if "nc" not in _NC_CACHE:
        _NC_CACHE["nc"] = build_program()
    return _NC_CACHE["nc"]


def run_sharded(inputs, trace=False):
    nc = get_program()
    in_maps = shard_inputs(**inputs)
    res = run_bass_kernel_spmd(nc, in_maps, list(range(N_CORES)), trace=trace)
    full = np.empty((BATCH, SEQ, D_MODEL), np.float32)
    for b in range(BATCH):
        acc = res.results[b * N_GROUPS]["out"].copy()
        for g in range(1, N_GROUPS):
            acc += res.results[b * N_GROUPS + g]["out"]
        full[b] = acc
    return full, res


def kernel(**inputs):
    out, _ = run_sharded(inputs, trace=False)
    return out


# revision 21
# speedup vs baseline: 1.2431x; 1.0282x over previous
"""Multi-head causal attention block on 8 Trainium2 NeuronCores.

Sharding: tensor-parallel over heads (4 groups of 4 heads) x data-parallel
over batch (2). Core c -> (batch b=c//4, head-group g=c%4). Each core
computes q/k/v projections for its head group, causal attention for its 4
heads, and a partial output projection; the host sums the 4 partials per
batch. All layout transposes are done host-side so the device does none.

Self-contained: hardcodes shapes for the 2x2048x2048, 16-head problem.
"""

import os
from contextlib import ExitStack

import numpy as np

import concourse.bass as bass
import concourse.tile as tile
from concourse import bacc, mybir
from concourse.bass import ds, ts
from concourse.bass_utils import run_bass_kernel_spmd

F32 = mybir.dt.float32
F32R = mybir.dt.float32r
ACTF = mybir.ActivationFunctionType

# Full-problem dims
BATCH = 2
SEQ = 2048
D_MODEL = 2048
NUM_HEADS = 16
HEAD_DIM = 128
N_CORES = 8
N_GROUPS = 4  # head-groups (tensor parallel)
DG = D_MODEL // N_GROUPS  # 512 = 4 heads per group
SCALE = 1.0 / float(np.sqrt(HEAD_DIM))

QB = 512  # q-block width in attention
KT = 128  # k-tile width (partition dim)

USE_F32R = os.environ.get("KERNEL_F32", "0") != "1"
MMDT = F32R if USE_F32R else F32


def _r(ap):
    """View a float32 DRAM AP as the matmul dtype for DMA into MMDT tiles."""
    return ap.bitcast(F32R) if USE_F32R else ap


def _mha_body(ctx, tc, aps, S, D, DGl):
    """Per-core kernel body.

    aps: dict of DRAM APs: xt [D,S], wqt/wkt/wvt [D,DGl], wot [DGl,D],
      bq/bk [128, DGl//128], bv [128, DGl], bo [128, D], masks [4,128,QB],
      ones [128,1], out [S,D].

    k^T and v stay resident in SBUF (written directly by the projection
    drains); only q^T round-trips through DRAM.
    """
    nc = tc.nc
    n_kd = D // 128  # contraction tiles over d_model
    n_sq = S // QB  # 512-wide attention q-blocks
    n_sk = S // KT  # 128-wide seq tiles
    n_dg = DGl // 128  # head tiles per group
    QB1 = 256  # phase-1 seq-slice width
    n_ns = S // QB1

    xt, wqt, wkt, wvt, wot = aps["xt"], aps["wqt"], aps["wkt"], aps["wvt"], aps["wot"]
    out = aps["out"]

    # DRAM scratch for v [S, DGl] (q^T and k^T stay resident in SBUF)
    dram = ctx.enter_context(tc.tile_pool(name="dram", bufs=1, space="DRAM"))
    v_d = dram.tile([S, DGl], F32, name="v_d")

    consts = ctx.enter_context(tc.tile_pool(name="consts", bufs=1))
    # dummy activation first: forces the ACT function-table DMA to queue
    # ahead of the bulk input loads (else every early PSUM drain stalls)
    warm = consts.tile([128, 1], F32, name="act_warm")
    nc.vector.memset(warm[:], 0.0)
    nc.scalar.activation(warm[:], warm[:], ACTF.Identity, bias=warm[:, 0:1])
    ones_sb = consts.tile([128, 1], MMDT, name="ones_sb")
    bq_sb = consts.tile([128, n_dg], F32, name="bq_sb")
    bk_sb = consts.tile([128, n_dg], F32, name="bk_sb")
    bv_sb = consts.tile([128, DGl], F32, name="bv_sb")
    masks_sb = consts.tile([128, 4 * QB], F32, name="masks_sb")

    # resident q^T / k^T per head: [p, s] = q^T/k^T[h*128+p, s]
    kv_pool = ctx.enter_context(tc.tile_pool(name="kv_res", bufs=1))
    kt_res = [
        kv_pool.tile([128, S], MMDT, tag=f"ktr{h}", name=f"kt_res{h}")
        for h in range(n_dg)
    ]
    qt_res = [
        kv_pool.tile([128, S], MMDT, tag=f"qtr{h}", name=f"qt_res{h}")
        for h in range(n_dg)
    ]

    # ---------------- Phase 1: q/k/v projections ----------------
    with (
        tc.tile_pool(name="wqkv", bufs=1) as wpool,
        tc.tile_pool(name="xt_pool", bufs=2) as xpool,
        tc.tile_pool(name="p1_stage", bufs=2) as stage,
        tc.tile_pool(name="p1_psum", bufs=4, space="PSUM") as psum1,
    ):
        # weights resident: w*_sb[p, k*DGl + f] = w*t[k*128+p, f]
        w_sbs = {
            wname: wpool.tile([128, n_kd * DGl], MMDT, name=f"{wname}_sb")
            for wname in ("wq", "wk", "wv")
        }

        def load_w(wname, wap):
            nc.sync.dma_start(
                w_sbs[wname][:].rearrange("p (k f) -> p k f", k=n_kd),
                _r(wap).rearrange("(k p) f -> p k f", p=128),
            )

        def load_w_mblock(wname, wap, m):
            nc.sync.dma_start(
                w_sbs[wname][:].rearrange(
                    "p (k g j) -> p k g j", k=n_kd, j=128
                )[:, :, m, :],
                _r(wap).rearrange("(k p) (g j) -> p k g j", p=128, j=128)[
                    :, :, m, :
                ],
            )

        def load_xt(ns):
            t = xpool.tile([128, n_kd * QB1], MMDT, tag="xt", name="xt_sb")
            nc.sync.dma_start(
                t[:].rearrange("p (k f) -> p k f", k=n_kd),
                _r(xt[:, ts(ns, QB1)]).rearrange("(k p) f -> p k f", p=128),
            )
            return t

        nc.sync.dma_start(ones_sb[:], _r(aps["ones"]))
        nc.sync.dma_start(bq_sb[:], aps["bq"])
        nc.sync.dma_start(bk_sb[:], aps["bk"])
        nc.sync.dma_start(bv_sb[:], aps["bv"])
        # k^T first: PE can start on wk+x0 while wq/wv still stream in
        load_w_mblock("wk", wkt, 0)
        # first x-slice in k-halves so the first accumulation starts sooner
        t = xpool.tile([128, n_kd * QB1], MMDT, tag="xt", name="xt_sb")
        half = n_kd // 2
        for hlf in range(2):
            nc.sync.dma_start(
                t[:, ds(hlf * half * QB1, half * QB1)].rearrange(
                    "p (k f) -> p k f", k=half
                ),
                _r(xt[ds(hlf * half * 128, half * 128), ts(0, QB1)]).rearrange(
                    "(k p) f -> p k f", p=128
                ),
            )
        g0 = t
        for m in range(1, n_dg):
            load_w_mblock("wk", wkt, m)
        g1 = load_xt(1)
        for m in range(n_dg):
            load_w_mblock("wq", wqt, m)
        load_w("wv", wvt)
        nc.sync.dma_start(
            masks_sb[:].rearrange("p (i f) -> p i f", i=4),
            aps["masks"].rearrange("i p f -> p i f"),
        )

        def do_proj_t(res, wname, b_sb, ns, xt_sb):
            # q^T/k^T [m hd-dims 128, QB1 seq] drains into resident tiles
            for m in range(n_dg):
                ps = psum1.tile([128, QB1], F32, tag="ps", name="ps_qk")
                for k in range(n_kd):
                    nc.tensor.matmul(
                        ps[:],
                        lhsT=w_sbs[wname][:, ds(k * DGl + m * 128, 128)],
                        rhs=xt_sb[:, ts(k, QB1)],
                        start=(k == 0),
                        stop=(k == n_kd - 1),
                    )
                nc.scalar.activation(
                    res[m][:, ts(ns, QB1)],
                    ps[:],
                    ACTF.Identity,
                    bias=b_sb[:, ds(m, 1)],
                )

        def do_v(ns, xt_sb):
            for msub in range(QB1 // 128):
                ps = psum1.tile([128, DGl], F32, tag="ps", name="ps_v")
                for k in range(n_kd):
                    nc.tensor.matmul(
                        ps[:],
                        lhsT=xt_sb[:, ds(k * QB1 + msub * 128, 128)],
                        rhs=w_sbs["wv"][:, ts(k, DGl)],
                        start=(k == 0),
                        stop=(k == n_kd - 1),
                    )
                st = stage.tile([128, DGl], F32, tag="v_st", name="v_st")
                nc.vector.tensor_add(st[:], ps[:], bv_sb[:])
                nc.sync.dma_start(
                    v_d[ds(ns * QB1 + msub * 128, 128), :], st[:]
                )

        # head group: k^T for slices 0-1 (no DMA drains), then q^T, then v
        for ns, g in ((0, g0), (1, g1)):
            do_proj_t(kt_res, "wk", bk_sb, ns, g)
        for ns, g in ((0, g0), (1, g1)):
            do_proj_t(qt_res, "wq", bq_sb, ns, g)
        for ns, g in ((0, g0), (1, g1)):
            do_v(ns, g)
        nxt = load_xt(2) if n_ns > 2 else None
        for ns in range(2, n_ns):
            xt_sb = nxt
            nxt = load_xt(ns + 1) if ns + 1 < n_ns else None
            do_proj_t(kt_res, "wk", bk_sb, ns, xt_sb)
            do_proj_t(qt_res, "wq", bq_sb, ns, xt_sb)
            do_v(ns, xt_sb)

    # ---------------- Phase 2: causal attention ----------------
    # ctx^T per head stays resident in SBUF for phase 3
    ctx_pool = ctx.enter_context(tc.tile_pool(name="ctx_pool", bufs=1))
    ctx_sbs = [
        ctx_pool.tile([128, S], MMDT, tag=f"ctx{h}", name=f"ctx_sb{h}")
        for h in range(n_dg)
    ]

    # wo stays resident; loaded mid-phase-2 so phase 3 starts hot
    wopool = ctx.enter_context(tc.tile_pool(name="wo_pool", bufs=1))
    wo_sb = wopool.tile([128, n_dg * D], MMDT, name="wo_sb")

    # phase-2/3-only constants live after phase-1 pools are freed
    p2consts = ctx.enter_context(tc.tile_pool(name="p2consts", bufs=1))
    bo_sb = p2consts.tile([128, D], F32, name="bo_sb")
    nc.sync.dma_start(bo_sb[:], aps["bo"])

    with (
        tc.tile_pool(name="v_pool", bufs=3) as vpool,
        tc.tile_pool(name="exp_pool", bufs=8) as epool,
        tc.tile_pool(name="lrec_pool", bufs=3) as lpool,
        tc.tile_pool(name="bc_pool", bufs=3) as bcpool,
        tc.tile_pool(name="ps_s", bufs=3, space="PSUM") as ps_s_pool,
        tc.tile_pool(name="ps_c", bufs=3, space="PSUM") as ps_c_pool,
        tc.tile_pool(name="ps_l", bufs=2, space="PSUM") as ps_l_pool,
    ):
        for h in range(n_dg):
            # v_sb[p, t*128+j] = v[t*128+p, h*128+j]; quarter DMAs so the
            # first q-blocks' PV can start before the whole head lands
            v_sb = vpool.tile([128, n_sk * 128], MMDT, tag="v", name="v_sb")
            nq = max(1, S // 512)
            for vq in range(nq):
                nc.sync.dma_start(
                    v_sb[:, ds(vq * 512, 512)].rearrange(
                        "p (t j) -> p t j", j=128
                    ),
                    _r(v_d[ds(vq * 512, 512), ts(h, 128)]).rearrange(
                        "(t p) j -> p t j", p=128
                    ),
                )
            if h == 1:
                # wo_sb[p, k*D + f] = wot[k*128+p, f] (phase-3 prefetch)
                nc.sync.dma_start(
                    wo_sb[:].rearrange("p (k f) -> p k f", k=n_dg),
                    _r(wot).rearrange("(k p) f -> p k f", p=128),
                )
            for qb in range(n_sq):
                n_kt = (qb + 1) * (QB // KT)  # causal: only k-tiles <= q
                ps_c = ps_c_pool.tile([128, QB], F32, tag="c", name="ps_c")
                ps_l = ps_l_pool.tile([1, QB], F32, tag="l", name="ps_l")
                diag0 = n_kt - (QB // KT)
                for kt in range(n_kt):
                    off = kt - diag0
                    # causal column restriction: diagonal tile off needs
                    # only cols >= off*128; keep moving dim >= 256 for
                    # full-rate f32r (so off=3 starts at 256, masked).
                    sc = 0 if off < 1 else (128 if off == 1 else 256)
                    w = QB - sc
                    ps_sc = ps_s_pool.tile([128, QB], F32, tag="s", name="ps_sc")
                    nc.tensor.matmul(
                        ps_sc[:, ds(sc, w)],
                        lhsT=kt_res[h][:, ts(kt, 128)],
                        rhs=qt_res[h][:, ds(qb * QB + sc, w)],
                        start=True,
                        stop=True,
                    )
                    if off >= 0:
                        # only the triangular block (plus, for off=3, the
                        # fully-invalid 128 cols kept for moving-dim>=256)
                        # needs masking; columns right of it are all-valid
                        msc = off * 128 if off < 3 else 256
                        mw = 128 if off < 3 else 256
                        nc.vector.tensor_add(
                            ps_sc[:, ds(msc, mw)],
                            ps_sc[:, ds(msc, mw)],
                            masks_sb[:, ds(off * QB + msc, mw)],
                        )
                    ex = epool.tile([128, QB], MMDT, tag="e", name="ex")
                    nc.scalar.activation(
                        ex[:, ds(sc, w)], ps_sc[:, ds(sc, w)], ACTF.Exp, scale=SCALE
                    )
                    nc.tensor.matmul(
                        ps_c[:, ds(sc, w)],
                        lhsT=v_sb[:, ts(kt, 128)],
                        rhs=ex[:, ds(sc, w)],
                        start=(kt == 0),
                        stop=(kt == n_kt - 1),
                        skip_group_check=True,
                    )
                    nc.tensor.matmul(
                        ps_l[:, ds(sc, w)],
                        lhsT=ones_sb[:],
                        rhs=ex[:, ds(sc, w)],
                        start=(kt == 0),
                        stop=(kt == n_kt - 1),
                        skip_group_check=True,
                    )
                rec = lpool.tile([1, QB], F32, tag="r", name="rec")
                nc.vector.reciprocal(rec[:], ps_l[:])
                bc = bcpool.tile([128, QB], F32, tag="bc", name="bc")
                nc.gpsimd.partition_broadcast(bc[:], rec[:])
                nc.vector.tensor_mul(
                    ctx_sbs[h][:, ts(qb, QB)], ps_c[:], bc[:]
                )

    # ---------------- Phase 3: output projection ----------------
    with (
        tc.tile_pool(name="o_stage", bufs=4) as ostage,
        tc.tile_pool(name="p3_psum", bufs=4, space="PSUM") as psum3,
    ):
        for m in range(n_sk):
            for n in range(D // QB):
                ps = psum3.tile([128, QB], F32, tag="o", name="ps_p3")
                for k in range(n_dg):
                    nc.tensor.matmul(
                        ps[:],
                        lhsT=ctx_sbs[k][:, ts(m, 128)],
                        rhs=wo_sb[:, ds(k * D + n * QB, QB)],
                        start=(k == 0),
                        stop=(k == n_dg - 1),
                    )
                ot = ostage.tile([128, QB], F32, tag="ot", name="ot")
                nc.vector.tensor_add(ot[:], ps[:], bo_sb[:, ts(n, QB)])
                nc.sync.dma_start(out[ts(m, 128), ts(n, QB)], ot[:])


def build_program(S=SEQ, D=D_MODEL, DGl=DG, enable_asserts=False):
    nc = bacc.Bacc(
        "TRN2",
        target_bir_lowering=False,
        debug=False,
        enable_asserts=enable_asserts,
        num_devices=N_CORES,
    )
    aps = {
        "xt": nc.dram_tensor("xt", [D, S], F32, kind="ExternalInput").ap(),
        "wqt": nc.dram_tensor("wqt", [D, DGl], F32, kind="ExternalInput").ap(),
        "wkt": nc.dram_tensor("wkt", [D, DGl], F32, kind="ExternalInput").ap(),
        "wvt": nc.dram_tensor("wvt", [D, DGl], F32, kind="ExternalInput").ap(),
        "wot": nc.dram_tensor("wot", [DGl, D], F32, kind="ExternalInput").ap(),
        "bq": nc.dram_tensor("bq", [128, DGl // 128], F32, kind="ExternalInput").ap(),
        "bk": nc.dram_tensor("bk", [128, DGl // 128], F32, kind="ExternalInput").ap(),
        "bv": nc.dram_tensor("bv", [128, DGl], F32, kind="ExternalInput").ap(),
        "bo": nc.dram_tensor("bo", [128, D], F32, kind="ExternalInput").ap(),
        "masks": nc.dram_tensor("masks", [4, 128, QB], F32, kind="ExternalInput").ap(),
        "ones": nc.dram_tensor("ones", [128, 1], F32, kind="ExternalInput").ap(),
        "out": nc.dram_tensor("out", [S, D], F32, kind="ExternalOutput").ap(),
    }
    with tile.TileContext(nc) as tc:
        with ExitStack() as ctx:
            _mha_body(ctx, tc, aps, S, D, DGl)
    nc.compile()
    return nc


def make_masks():
    """Additive causal masks: 0 where k<=q, -1e30 where masked."""
    i = np.arange(4)[:, None, None]
    p = np.arange(128)[None, :, None]
    f = np.arange(QB)[None, None, :]
    keep = (i * 128 + p) <= f
    return np.where(keep, 0.0, -1e30).astype(np.float32)


def shard_inputs(x, wq, bq, wk, bk, wv, bv, wo, bo):
    """Build the 8 per-core input maps (host-side layout prep)."""
    masks = make_masks()
    xts = [np.ascontiguousarray(np.asarray(x[b], np.float32).T) for b in range(BATCH)]
    bo_bc = np.ascontiguousarray(
        np.broadcast_to(np.asarray(bo, np.float32), (128, D_MODEL))
    )
    bo_zero = np.zeros((128, D_MODEL), np.float32)
    in_maps = []
    for c in range(N_CORES):
        b, g = divmod(c, N_GROUPS)
        sl = slice(g * DG, (g + 1) * DG)
        in_maps.append(
            {
                "xt": xts[b],
                "wqt": np.ascontiguousarray(np.asarray(wq, np.float32)[sl].T),
                "wkt": np.ascontiguousarray(np.asarray(wk, np.float32)[sl].T),
                "wvt": np.ascontiguousarray(np.asarray(wv, np.float32)[sl].T),
                "wot": np.ascontiguousarray(np.asarray(wo, np.float32)[:, sl].T),
                "bq": np.ascontiguousarray(
                    np.asarray(bq, np.float32)[sl].reshape(-1, 128).T
                ),
                "bk": np.ascontiguousarray(
                    np.asarray(bk, np.float32)[sl].reshape(-1, 128).T
                ),
                "bv": np.ascontiguousarray(
                    np.broadcast_to(np.asarray(bv, np.float32)[sl], (128, DG))
                ),
                "bo": bo_bc if g == 0 else bo_zero,
                "masks": masks,
                "ones": np.ones((128, 1), np.float32),
            }
        )
    return in_maps


_NC_CACHE = {}


def get_program():
    if "nc" not in _NC_CACHE:
        _NC_CACHE["nc"] = build_program()
    return _NC_CACHE["nc"]


def run_sharded(inputs, trace=False):
    nc = get_program()
    in_maps = shard_inputs(**inputs)
    res = run_bass_kernel_spmd(nc, in_maps, list(range(N_CORES)), trace=trace)
    full = np.empty((BATCH, SEQ, D_MODEL), np.float32)
    for b in range(BATCH):
        acc = res.results[b * N_GROUPS]["out"].copy()
        for g in range(1, N_GROUPS):
            acc += res.results[b * N_GROUPS + g]["out"]
        full[b] = acc
    return full, res


def kernel(**inputs):
    out, _ = run_sharded(inputs, trace=False)
    return out


# revision 22
# speedup vs baseline: 1.2779x; 1.0280x over previous
"""Multi-head causal attention block on 8 Trainium2 NeuronCores.

Sharding: tensor-parallel over heads (4 groups of 4 heads) x data-parallel
over batch (2). Core c -> (batch b=c//4, head-group g=c%4). Each core
computes q/k/v projections for its head group, causal attention for its 4
heads, and a partial output projection; the host sums the 4 partials per
batch. All layout transposes are done host-side so the device does none.

Self-contained: hardcodes shapes for the 2x2048x2048, 16-head problem.
"""

import os
from contextlib import ExitStack

import numpy as np

import concourse.bass as bass
import concourse.tile as tile
from concourse import bacc, mybir
from concourse.bass import ds, ts
from concourse.bass_utils import run_bass_kernel_spmd

F32 = mybir.dt.float32
F32R = mybir.dt.float32r
ACTF = mybir.ActivationFunctionType

# Full-problem dims
BATCH = 2
SEQ = 2048
D_MODEL = 2048
NUM_HEADS = 16
HEAD_DIM = 128
N_CORES = 8
N_GROUPS = 4  # head-groups (tensor parallel)
DG = D_MODEL // N_GROUPS  # 512 = 4 heads per group
SCALE = 1.0 / float(np.sqrt(HEAD_DIM))

QB = 512  # q-block width in attention
KT = 128  # k-tile width (partition dim)

USE_F32R = os.environ.get("KERNEL_F32", "0") != "1"
MMDT = F32R if USE_F32R else F32


def _r(ap):
    """View a float32 DRAM AP as the matmul dtype for DMA into MMDT tiles."""
    return ap.bitcast(F32R) if USE_F32R else ap


def _mha_body(ctx, tc, aps, S, D, DGl):
    """Per-core kernel body.

    aps: dict of DRAM APs: xt [D,S], wqt/wkt/wvt [D,DGl], wot [DGl,D],
      bq/bk [128, DGl//128], bv [128, DGl], bo [128, D], masks [4,128,QB],
      ones [128,1], out [S,D].

    k^T and v stay resident in SBUF (written directly by the projection
    drains); only q^T round-trips through DRAM.
    """
    nc = tc.nc
    n_kd = D // 128  # contraction tiles over d_model
    n_sq = S // QB  # 512-wide attention q-blocks
    n_sk = S // KT  # 128-wide seq tiles
    n_dg = DGl // 128  # head tiles per group
    QB1 = 256  # phase-1 seq-slice width
    n_ns = S // QB1

    xt, wqt, wkt, wvt, wot = aps["xt"], aps["wqt"], aps["wkt"], aps["wvt"], aps["wot"]
    out = aps["out"]

    # DRAM scratch for v [S, DGl] (q^T and k^T stay resident in SBUF)
    dram = ctx.enter_context(tc.tile_pool(name="dram", bufs=1, space="DRAM"))
    v_d = dram.tile([S, DGl], F32, name="v_d")

    consts = ctx.enter_context(tc.tile_pool(name="consts", bufs=1))
    # dummy activation first: forces the ACT function-table DMA to queue
    # ahead of the bulk input loads (else every early PSUM drain stalls)
    warm = consts.tile([128, 1], F32, name="act_warm")
    nc.vector.memset(warm[:], 0.0)
    nc.scalar.activation(warm[:], warm[:], ACTF.Identity, bias=warm[:, 0:1])
    ones_sb = consts.tile([128, 1], MMDT, name="ones_sb")
    bq_sb = consts.tile([128, n_dg], F32, name="bq_sb")
    bk_sb = consts.tile([128, n_dg], F32, name="bk_sb")
    bv_sb = consts.tile([128, DGl], F32, name="bv_sb")
    masks_sb = consts.tile([128, 4 * QB], F32, name="masks_sb")

    # resident q^T / k^T per head: [p, s] = q^T/k^T[h*128+p, s]
    kv_pool = ctx.enter_context(tc.tile_pool(name="kv_res", bufs=1))
    kt_res = [
        kv_pool.tile([128, S], MMDT, tag=f"ktr{h}", name=f"kt_res{h}")
        for h in range(n_dg)
    ]
    qt_res = [
        kv_pool.tile([128, S], MMDT, tag=f"qtr{h}", name=f"qt_res{h}")
        for h in range(n_dg)
    ]

    # ---------------- Phase 1: q/k/v projections ----------------
    with (
        tc.tile_pool(name="wqkv", bufs=1) as wpool,
        tc.tile_pool(name="xt_pool", bufs=2) as xpool,
        tc.tile_pool(name="p1_stage", bufs=2) as stage,
        tc.tile_pool(name="p1_psum", bufs=4, space="PSUM") as psum1,
    ):
        # weights resident: w*_sb[p, k*DGl + f] = w*t[k*128+p, f]
        w_sbs = {
            wname: wpool.tile([128, n_kd * DGl], MMDT, name=f"{wname}_sb")
            for wname in ("wq", "wk", "wv")
        }

        def load_w(wname, wap):
            nc.sync.dma_start(
                w_sbs[wname][:].rearrange("p (k f) -> p k f", k=n_kd),
                _r(wap).rearrange("(k p) f -> p k f", p=128),
            )

        def load_w_mblock(wname, wap, m):
            nc.sync.dma_start(
                w_sbs[wname][:].rearrange(
                    "p (k g j) -> p k g j", k=n_kd, j=128
                )[:, :, m, :],
                _r(wap).rearrange("(k p) (g j) -> p k g j", p=128, j=128)[
                    :, :, m, :
                ],
            )

        def load_xt(ns):
            # two k-half DMAs: the slice's first k-accumulations can start
            # as soon as the first half lands
            t = xpool.tile([128, n_kd * QB1], MMDT, tag="xt", name="xt_sb")
            half = n_kd // 2
            for hlf in range(2):
                nc.sync.dma_start(
                    t[:, ds(hlf * half * QB1, half * QB1)].rearrange(
                        "p (k f) -> p k f", k=half
                    ),
                    _r(
                        xt[ds(hlf * half * 128, half * 128), ts(ns, QB1)]
                    ).rearrange("(k p) f -> p k f", p=128),
                )
            return t

        nc.sync.dma_start(ones_sb[:], _r(aps["ones"]))
        nc.sync.dma_start(bq_sb[:], aps["bq"])
        nc.sync.dma_start(bk_sb[:], aps["bk"])
        nc.sync.dma_start(bv_sb[:], aps["bv"])
        # k^T first: PE can start on wk+x0 while wq/wv still stream in
        load_w_mblock("wk", wkt, 0)
        g0 = load_xt(0)
        for m in range(1, n_dg):
            load_w_mblock("wk", wkt, m)
        g1 = load_xt(1)
        for m in range(n_dg):
            load_w_mblock("wq", wqt, m)
        load_w("wv", wvt)
        nc.sync.dma_start(
            masks_sb[:].rearrange("p (i f) -> p i f", i=4),
            aps["masks"].rearrange("i p f -> p i f"),
        )

        def do_proj_t(res, wname, b_sb, ns, xt_sb):
            # q^T/k^T [m hd-dims 128, QB1 seq] drains into resident tiles
            for m in range(n_dg):
                ps = psum1.tile([128, QB1], F32, tag="ps", name="ps_qk")
                for k in range(n_kd):
                    nc.tensor.matmul(
                        ps[:],
                        lhsT=w_sbs[wname][:, ds(k * DGl + m * 128, 128)],
                        rhs=xt_sb[:, ts(k, QB1)],
                        start=(k == 0),
                        stop=(k == n_kd - 1),
                    )
                nc.scalar.activation(
                    res[m][:, ts(ns, QB1)],
                    ps[:],
                    ACTF.Identity,
                    bias=b_sb[:, ds(m, 1)],
                )

        def do_v(ns, xt_sb):
            for msub in range(QB1 // 128):
                ps = psum1.tile([128, DGl], F32, tag="ps", name="ps_v")
                for k in range(n_kd):
                    nc.tensor.matmul(
                        ps[:],
                        lhsT=xt_sb[:, ds(k * QB1 + msub * 128, 128)],
                        rhs=w_sbs["wv"][:, ts(k, DGl)],
                        start=(k == 0),
                        stop=(k == n_kd - 1),
                    )
                st = stage.tile([128, DGl], F32, tag="v_st", name="v_st")
                nc.vector.tensor_add(st[:], ps[:], bv_sb[:])
                nc.sync.dma_start(
                    v_d[ds(ns * QB1 + msub * 128, 128), :], st[:]
                )

        # head group: k^T for slices 0-1 (no DMA drains), then q^T, then v
        for ns, g in ((0, g0), (1, g1)):
            do_proj_t(kt_res, "wk", bk_sb, ns, g)
        for ns, g in ((0, g0), (1, g1)):
            do_proj_t(qt_res, "wq", bq_sb, ns, g)
        for ns, g in ((0, g0), (1, g1)):
            do_v(ns, g)
        nxt = load_xt(2) if n_ns > 2 else None
        for ns in range(2, n_ns):
            xt_sb = nxt
            nxt = load_xt(ns + 1) if ns + 1 < n_ns else None
            do_proj_t(kt_res, "wk", bk_sb, ns, xt_sb)
            do_proj_t(qt_res, "wq", bq_sb, ns, xt_sb)
            do_v(ns, xt_sb)

    # ---------------- Phase 2: causal attention ----------------
    # ctx^T per head stays resident in SBUF for phase 3
    ctx_pool = ctx.enter_context(tc.tile_pool(name="ctx_pool", bufs=1))
    ctx_sbs = [
        ctx_pool.tile([128, S], MMDT, tag=f"ctx{h}", name=f"ctx_sb{h}")
        for h in range(n_dg)
    ]

    # wo stays resident; loaded mid-phase-2 so phase 3 starts hot
    wopool = ctx.enter_context(tc.tile_pool(name="wo_pool", bufs=1))
    wo_sb = wopool.tile([128, n_dg * D], MMDT, name="wo_sb")

    # phase-2/3-only constants live after phase-1 pools are freed
    p2consts = ctx.enter_context(tc.tile_pool(name="p2consts", bufs=1))
    bo_sb = p2consts.tile([128, D], F32, name="bo_sb")
    nc.sync.dma_start(bo_sb[:], aps["bo"])

    with (
        tc.tile_pool(name="v_pool", bufs=3) as vpool,
        tc.tile_pool(name="exp_pool", bufs=8) as epool,
        tc.tile_pool(name="lrec_pool", bufs=3) as lpool,
        tc.tile_pool(name="bc_pool", bufs=3) as bcpool,
        tc.tile_pool(name="ps_s", bufs=3, space="PSUM") as ps_s_pool,
        tc.tile_pool(name="ps_c", bufs=3, space="PSUM") as ps_c_pool,
        tc.tile_pool(name="ps_l", bufs=2, space="PSUM") as ps_l_pool,
    ):
        for h in range(n_dg):
            # v_sb[p, t*128+j] = v[t*128+p, h*128+j]; quarter DMAs so the
            # first q-blocks' PV can start before the whole head lands
            v_sb = vpool.tile([128, n_sk * 128], MMDT, tag="v", name="v_sb")
            nq = max(1, S // 512)
            for vq in range(nq):
                nc.sync.dma_start(
                    v_sb[:, ds(vq * 512, 512)].rearrange(
                        "p (t j) -> p t j", j=128
                    ),
                    _r(v_d[ds(vq * 512, 512), ts(h, 128)]).rearrange(
                        "(t p) j -> p t j", p=128
                    ),
                )
            if h == 1:
                # wo_sb[p, k*D + f] = wot[k*128+p, f] (phase-3 prefetch)
                nc.sync.dma_start(
                    wo_sb[:].rearrange("p (k f) -> p k f", k=n_dg),
                    _r(wot).rearrange("(k p) f -> p k f", p=128),
                )
            for qb in range(n_sq):
                n_kt = (qb + 1) * (QB // KT)  # causal: only k-tiles <= q
                ps_c = ps_c_pool.tile([128, QB], F32, tag="c", name="ps_c")
                ps_l = ps_l_pool.tile([1, QB], F32, tag="l", name="ps_l")
                diag0 = n_kt - (QB // KT)
                for kt in range(n_kt):
                    off = kt - diag0
                    # causal column restriction: diagonal tile off needs
                    # only cols >= off*128; keep moving dim >= 256 for
                    # full-rate f32r (so off=3 starts at 256, masked).
                    sc = 0 if off < 1 else (128 if off == 1 else 256)
                    w = QB - sc
                    ps_sc = ps_s_pool.tile([128, QB], F32, tag="s", name="ps_sc")
                    nc.tensor.matmul(
                        ps_sc[:, ds(sc, w)],
                        lhsT=kt_res[h][:, ts(kt, 128)],
                        rhs=qt_res[h][:, ds(qb * QB + sc, w)],
                        start=True,
                        stop=True,
                    )
                    if off >= 0:
                        # only the triangular block (plus, for off=3, the
                        # fully-invalid 128 cols kept for moving-dim>=256)
                        # needs masking; columns right of it are all-valid
                        msc = off * 128 if off < 3 else 256
                        mw = 128 if off < 3 else 256
                        nc.vector.tensor_add(
                            ps_sc[:, ds(msc, mw)],
                            ps_sc[:, ds(msc, mw)],
                            masks_sb[:, ds(off * QB + msc, mw)],
                        )
                    ex = epool.tile([128, QB], MMDT, tag="e", name="ex")
                    nc.scalar.activation(
                        ex[:, ds(sc, w)], ps_sc[:, ds(sc, w)], ACTF.Exp, scale=SCALE
                    )
                    nc.tensor.matmul(
                        ps_c[:, ds(sc, w)],
                        lhsT=v_sb[:, ts(kt, 128)],
                        rhs=ex[:, ds(sc, w)],
                        start=(kt == 0),
                        stop=(kt == n_kt - 1),
                        skip_group_check=True,
                    )
                    nc.tensor.matmul(
                        ps_l[:, ds(sc, w)],
                        lhsT=ones_sb[:],
                        rhs=ex[:, ds(sc, w)],
                        start=(kt == 0),
                        stop=(kt == n_kt - 1),
                        skip_group_check=True,
                    )
                rec = lpool.tile([1, QB], F32, tag="r", name="rec")
                nc.vector.reciprocal(rec[:], ps_l[:])
                bc = bcpool.tile([128, QB], F32, tag="bc", name="bc")
                nc.gpsimd.partition_broadcast(bc[:], rec[:])
                nc.vector.tensor_mul(
                    ctx_sbs[h][:, ts(qb, QB)], ps_c[:], bc[:]
                )

    # ---------------- Phase 3: output projection ----------------
    with (
        tc.tile_pool(name="o_stage", bufs=4) as ostage,
        tc.tile_pool(name="p3_psum", bufs=4, space="PSUM") as psum3,
    ):
        for m in range(n_sk):
            for n in range(D // QB):
                ps = psum3.tile([128, QB], F32, tag="o", name="ps_p3")
                for k in range(n_dg):
                    nc.tensor.matmul(
                        ps[:],
                        lhsT=ctx_sbs[k][:, ts(m, 128)],
                        rhs=wo_sb[:, ds(k * D + n * QB, QB)],
                        start=(k == 0),
                        stop=(k == n_dg - 1),
                    )
                ot = ostage.tile([128, QB], F32, tag="ot", name="ot")
                nc.vector.tensor_add(ot[:], ps[:], bo_sb[:, ts(n, QB)])
                nc.sync.dma_start(out[ts(m, 128), ts(n, QB)], ot[:])


def build_program(S=SEQ, D=D_MODEL, DGl=DG, enable_asserts=False):
    nc = bacc.Bacc(
        "TRN2",
        target_bir_lowering=False,
        debug=False,
        enable_asserts=enable_asserts,
        num_devices=N_CORES,
    )
    aps = {
        "xt": nc.dram_tensor("xt", [D, S], F32, kind="ExternalInput").ap(),
        "wqt": nc.dram_tensor("wqt", [D, DGl], F32, kind="ExternalInput").ap(),
        "wkt": nc.dram_tensor("wkt", [D, DGl], F32, kind="ExternalInput").ap(),
        "wvt": nc.dram_tensor("wvt", [D, DGl], F32, kind="ExternalInput").ap(),
        "wot": nc.dram_tensor("wot", [DGl, D], F32, kind="ExternalInput").ap(),
        "bq": nc.dram_tensor("bq", [128, DGl // 128], F32, kind="ExternalInput").ap(),
        "bk": nc.dram_tensor("bk", [128, DGl // 128], F32, kind="ExternalInput").ap(),
        "bv": nc.dram_tensor("bv", [128, DGl], F32, kind="ExternalInput").ap(),
        "bo": nc.dram_tensor("bo", [128, D], F32, kind="ExternalInput").ap(),
        "masks": nc.dram_tensor("masks", [4, 128, QB], F32, kind="ExternalInput").ap(),
        "ones": nc.dram_tensor("ones", [128, 1], F32, kind="ExternalInput").ap(),
        "out": nc.dram_tensor("out", [S, D], F32, kind="ExternalOutput").ap(),
    }
    with tile.TileContext(nc) as tc:
        with ExitStack() as ctx:
            _mha_body(ctx, tc, aps, S, D, DGl)
    nc.compile()
    return nc


def make_masks():
    """Additive causal masks: 0 where k<=q, -1e30 where masked."""
    i = np.arange(4)[:, None, None]
    p = np.arange(128)[None, :, None]
    f = np.arange(QB)[None, None, :]
    keep = (i * 128 + p) <= f
    return np.where(keep, 0.0, -1e30).astype(np.float32)


def shard_inputs(x, wq, bq, wk, bk, wv, bv, wo, bo):
    """Build the 8 per-core input maps (host-side layout prep)."""
    masks = make_masks()
    xts = [np.ascontiguousarray(np.asarray(x[b], np.float32).T) for b in range(BATCH)]
    bo_bc = np.ascontiguousarray(
        np.broadcast_to(np.asarray(bo, np.float32), (128, D_MODEL))
    )
    bo_zero = np.zeros((128, D_MODEL), np.float32)
    in_maps = []
    for c in range(N_CORES):
        b, g = divmod(c, N_GROUPS)
        sl = slice(g * DG, (g + 1) * DG)
        in_maps.append(
            {
                "xt": xts[b],
                "wqt": np.ascontiguousarray(np.asarray(wq, np.float32)[sl].T),
                "wkt": np.ascontiguousarray(np.asarray(wk, np.float32)[sl].T),
                "wvt": np.ascontiguousarray(np.asarray(wv, np.float32)[sl].T),
                "wot": np.ascontiguousarray(np.asarray(wo, np.float32)[:, sl].T),
                "bq": np.ascontiguousarray(
                    np.asarray(bq, np.float32)[sl].reshape(-1, 128).T
                ),
                "bk": np.ascontiguousarray(
                    np.asarray(bk, np.float32)[sl].reshape(-1, 128).T
                ),
                "bv": np.ascontiguousarray(
                    np.broadcast_to(np.asarray(bv, np.float32)[sl], (128, DG))
                ),
                "bo": bo_bc if g == 0 else bo_zero,
                "masks": masks,
                "ones": np.ones((128, 1), np.float32),
            }
        )
    return in_maps


_NC_CACHE = {}


def get_program():
    if "nc" not in _NC_CACHE:
        _NC_CACHE["nc"] = build_program()
    return _NC_CACHE["nc"]


def run_sharded(inputs, trace=False):
    nc = get_program()
    in_maps = shard_inputs(**inputs)
    res = run_bass_kernel_spmd(nc, in_maps, list(range(N_CORES)), trace=trace)
    full = np.empty((BATCH, SEQ, D_MODEL), np.float32)
    for b in range(BATCH):
        acc = res.results[b * N_GROUPS]["out"].copy()
        for g in range(1, N_GROUPS):
            acc += res.results[b * N_GROUPS + g]["out"]
        full[b] = acc
    return full, res


def kernel(**inputs):
    out, _ = run_sharded(inputs, trace=False)
    return out


# revision 26
# speedup vs baseline: 1.3211x; 1.0337x over previous
"""Multi-head causal attention block on 8 Trainium2 NeuronCores.

Sharding: tensor-parallel over heads (4 groups of 4 heads) x data-parallel
over batch (2). Core c -> (batch b=c//4, head-group g=c%4). Each core
computes q/k/v projections for its head group, causal attention for its 4
heads, and a partial output projection; the host sums the 4 partials per
batch. All layout transposes are done host-side so the device does none.

Self-contained: hardcodes shapes for the 2x2048x2048, 16-head problem.
"""

import os
from contextlib import ExitStack

import numpy as np

import concourse.bass as bass
import concourse.tile as tile
from concourse import bacc, mybir
from concourse.bass import ds, ts
from concourse.bass_utils import run_bass_kernel_spmd

F32 = mybir.dt.float32
F32R = mybir.dt.float32r
ACTF = mybir.ActivationFunctionType

# Full-problem dims
BATCH = 2
SEQ = 2048
D_MODEL = 2048
NUM_HEADS = 16
HEAD_DIM = 128
N_CORES = 8
N_GROUPS = 4  # head-groups (tensor parallel)
DG = D_MODEL // N_GROUPS  # 512 = 4 heads per group
SCALE = 1.0 / float(np.sqrt(HEAD_DIM))

QB = 512  # q-block width in attention
KT = 128  # k-tile width (partition dim)

USE_F32R = os.environ.get("KERNEL_F32", "0") != "1"
MMDT = F32R if USE_F32R else F32


def _r(ap):
    """View a float32 DRAM AP as the matmul dtype for DMA into MMDT tiles."""
    return ap.bitcast(F32R) if USE_F32R else ap


def _mha_body(ctx, tc, aps, S, D, DGl):
    """Per-core kernel body.

    aps: dict of DRAM APs: xt [D,S], wqt/wkt/wvt [D,DGl], wot [DGl,D],
      bq/bk [128, DGl//128], bv [128, DGl], bo [128, D], masks [4,128,QB],
      ones [128,1], out [S,D].

    k^T and v stay resident in SBUF (written directly by the projection
    drains); only q^T round-trips through DRAM.
    """
    nc = tc.nc
    n_kd = D // 128  # contraction tiles over d_model
    n_sq = S // QB  # 512-wide attention q-blocks
    n_sk = S // KT  # 128-wide seq tiles
    n_dg = DGl // 128  # head tiles per group
    QB1 = 256  # phase-1 seq-slice width
    n_ns = S // QB1

    xt, wqt, wkt, wvt, wot = aps["xt"], aps["wqt"], aps["wkt"], aps["wvt"], aps["wot"]
    out = aps["out"]

    # DRAM scratch for v [S, DGl] (q^T and k^T stay resident in SBUF)
    dram = ctx.enter_context(tc.tile_pool(name="dram", bufs=1, space="DRAM"))
    v_d = dram.tile([S, DGl], F32, name="v_d")

    consts = ctx.enter_context(tc.tile_pool(name="consts", bufs=1))
    # dummy activation first: forces the ACT function-table DMA to queue
    # ahead of the bulk input loads (else every early PSUM drain stalls)
    warm = consts.tile([128, 1], F32, name="act_warm")
    nc.vector.memset(warm[:], 0.0)
    nc.scalar.activation(warm[:], warm[:], ACTF.Identity, bias=warm[:, 0:1])
    ones_sb = consts.tile([128, 1], MMDT, name="ones_sb")
    bq_sb = consts.tile([128, n_dg], F32, name="bq_sb")
    bk_sb = consts.tile([128, n_dg], F32, name="bk_sb")
    bv_sb = consts.tile([128, DGl], F32, name="bv_sb")
    masks_sb = consts.tile([128, 4 * QB], F32, name="masks_sb")

    # resident q^T / k^T per head: [p, s] = q^T/k^T[h*128+p, s]
    kv_pool = ctx.enter_context(tc.tile_pool(name="kv_res", bufs=1))
    kt_res = [
        kv_pool.tile([128, S], MMDT, tag=f"ktr{h}", name=f"kt_res{h}")
        for h in range(n_dg)
    ]
    qt_res = [
        kv_pool.tile([128, S], MMDT, tag=f"qtr{h}", name=f"qt_res{h}")
        for h in range(n_dg)
    ]

    # ---------------- Phase 1: q/k/v projections ----------------
    with (
        tc.tile_pool(name="wqkv", bufs=1) as wpool,
        tc.tile_pool(name="xt_pool", bufs=2) as xpool,
        tc.tile_pool(name="p1_stage", bufs=2) as stage,
        tc.tile_pool(name="p1_psum", bufs=4, space="PSUM") as psum1,
    ):
        # weights resident: w*_sb[p, k*DGl + f] = w*t[k*128+p, f]
        w_sbs = {
            wname: wpool.tile([128, n_kd * DGl], MMDT, name=f"{wname}_sb")
            for wname in ("wq", "wk", "wv")
        }

        def load_w(wname, wap):
            nc.sync.dma_start(
                w_sbs[wname][:].rearrange("p (k f) -> p k f", k=n_kd),
                _r(wap).rearrange("(k p) f -> p k f", p=128),
            )

        def load_w_mblock(wname, wap, m):
            nc.sync.dma_start(
                w_sbs[wname][:].rearrange(
                    "p (k g j) -> p k g j", k=n_kd, j=128
                )[:, :, m, :],
                _r(wap).rearrange("(k p) (g j) -> p k g j", p=128, j=128)[
                    :, :, m, :
                ],
            )

        def load_xt(ns):
            # two k-half DMAs: the slice's first k-accumulations can start
            # as soon as the first half lands
            t = xpool.tile([128, n_kd * QB1], MMDT, tag="xt", name="xt_sb")
            half = n_kd // 2
            for hlf in range(2):
                nc.sync.dma_start(
                    t[:, ds(hlf * half * QB1, half * QB1)].rearrange(
                        "p (k f) -> p k f", k=half
                    ),
                    _r(
                        xt[ds(hlf * half * 128, half * 128), ts(ns, QB1)]
                    ).rearrange("(k p) f -> p k f", p=128),
                )
            return t

        nc.sync.dma_start(ones_sb[:], _r(aps["ones"]))
        nc.sync.dma_start(bq_sb[:], aps["bq"])
        nc.sync.dma_start(bk_sb[:], aps["bk"])
        nc.sync.dma_start(bv_sb[:], aps["bv"])
        # k^T first: PE can start on wk+x0 while wq/wv still stream in
        load_w_mblock("wk", wkt, 0)
        g0 = load_xt(0)
        for m in range(1, n_dg):
            load_w_mblock("wk", wkt, m)
        g1 = load_xt(1)
        for m in range(n_dg):
            load_w_mblock("wq", wqt, m)
        load_w("wv", wvt)
        nc.sync.dma_start(
            masks_sb[:].rearrange("p (i f) -> p i f", i=4),
            aps["masks"].rearrange("i p f -> p i f"),
        )

        def do_proj_t(res, wname, b_sb, ns, xt_sb):
            # q^T/k^T [m hd-dims 128, QB1 seq] drains into resident tiles
            for m in range(n_dg):
                ps = psum1.tile([128, QB1], F32, tag="ps", name="ps_qk")
                for k in range(n_kd):
                    nc.tensor.matmul(
                        ps[:],
                        lhsT=w_sbs[wname][:, ds(k * DGl + m * 128, 128)],
                        rhs=xt_sb[:, ts(k, QB1)],
                        start=(k == 0),
                        stop=(k == n_kd - 1),
                    )
                nc.scalar.activation(
                    res[m][:, ts(ns, QB1)],
                    ps[:],
                    ACTF.Identity,
                    bias=b_sb[:, ds(m, 1)],
                )

        def do_v(ns, xt_sb):
            for msub in range(QB1 // 128):
                ps = psum1.tile([128, DGl], F32, tag="ps", name="ps_v")
                for k in range(n_kd):
                    nc.tensor.matmul(
                        ps[:],
                        lhsT=xt_sb[:, ds(k * QB1 + msub * 128, 128)],
                        rhs=w_sbs["wv"][:, ts(k, DGl)],
                        start=(k == 0),
                        stop=(k == n_kd - 1),
                    )
                st = stage.tile([128, DGl], F32, tag="v_st", name="v_st")
                nc.vector.tensor_add(st[:], ps[:], bv_sb[:])
                nc.sync.dma_start(
                    v_d[ds(ns * QB1 + msub * 128, 128), :], st[:]
                )

        # head group: k^T for slices 0-1 (no DMA drains), then q^T, then v
        for ns, g in ((0, g0), (1, g1)):
            do_proj_t(kt_res, "wk", bk_sb, ns, g)
        for ns, g in ((0, g0), (1, g1)):
            do_proj_t(qt_res, "wq", bq_sb, ns, g)
        for ns, g in ((0, g0), (1, g1)):
            do_v(ns, g)
        nxt = load_xt(2) if n_ns > 2 else None
        for ns in range(2, n_ns):
            xt_sb = nxt
            nxt = load_xt(ns + 1) if ns + 1 < n_ns else None
            do_proj_t(kt_res, "wk", bk_sb, ns, xt_sb)
            do_proj_t(qt_res, "wq", bq_sb, ns, xt_sb)
            do_v(ns, xt_sb)

    # ---------------- Phase 2: causal attention ----------------
    # ctx^T per head stays resident in SBUF for phase 3
    ctx_pool = ctx.enter_context(tc.tile_pool(name="ctx_pool", bufs=1))
    ctx_sbs = [
        ctx_pool.tile([128, S], MMDT, tag=f"ctx{h}", name=f"ctx_sb{h}")
        for h in range(n_dg)
    ]

    # wo stays resident; loaded mid-phase-2 so phase 3 starts hot
    wopool = ctx.enter_context(tc.tile_pool(name="wo_pool", bufs=1))
    wo_sb = wopool.tile([128, n_dg * D], MMDT, name="wo_sb")

    # phase-2/3-only constants live after phase-1 pools are freed
    p2consts = ctx.enter_context(tc.tile_pool(name="p2consts", bufs=1))
    bo_sb = p2consts.tile([128, D], F32, name="bo_sb")
    nc.sync.dma_start(bo_sb[:], aps["bo"])

    with (
        tc.tile_pool(name="v_pool", bufs=3) as vpool,
        tc.tile_pool(name="exp_pool", bufs=8) as epool,
        tc.tile_pool(name="lrec_pool", bufs=3) as lpool,
        tc.tile_pool(name="bc_pool", bufs=3) as bcpool,
        tc.tile_pool(name="ps_s", bufs=4, space="PSUM") as ps_s_pool,
        tc.tile_pool(name="ps_c", bufs=3, space="PSUM") as ps_c_pool,
        tc.tile_pool(name="ps_l", bufs=1, space="PSUM") as ps_l_pool,
    ):
        for h in range(n_dg):
            # v_sb[p, t*128+j] = v[t*128+p, h*128+j]; quarter DMAs so the
            # first q-blocks' PV can start before the whole head lands
            v_sb = vpool.tile([128, n_sk * 128], MMDT, tag="v", name="v_sb")
            nq = max(1, S // 512)
            for vq in range(nq):
                nc.sync.dma_start(
                    v_sb[:, ds(vq * 512, 512)].rearrange(
                        "p (t j) -> p t j", j=128
                    ),
                    _r(v_d[ds(vq * 512, 512), ts(h, 128)]).rearrange(
                        "(t p) j -> p t j", p=128
                    ),
                )
            if h == 1:
                # wo_sb[p, k*D + f] = wot[k*128+p, f] (phase-3 prefetch)
                nc.sync.dma_start(
                    wo_sb[:].rearrange("p (k f) -> p k f", k=n_dg),
                    _r(wot).rearrange("(k p) f -> p k f", p=128),
                )
            for qb in range(n_sq):
                n_kt = (qb + 1) * (QB // KT)  # causal: only k-tiles <= q
                ps_c = ps_c_pool.tile([128, QB], F32, tag="c", name="ps_c")
                ps_l = ps_l_pool.tile([1, QB], F32, tag="l", name="ps_l")
                diag0 = n_kt - (QB // KT)
                for kt in range(n_kt):
                    off = kt - diag0
                    # causal column restriction: diagonal tile off needs
                    # only cols >= off*128; keep moving dim >= 256 for
                    # full-rate f32r (so off=3 starts at 256, masked).
                    sc = 0 if off < 1 else (128 if off == 1 else 256)
                    w = QB - sc
                    ps_sc = ps_s_pool.tile([128, QB], F32, tag="s", name="ps_sc")
                    nc.tensor.matmul(
                        ps_sc[:, ds(sc, w)],
                        lhsT=kt_res[h][:, ts(kt, 128)],
                        rhs=qt_res[h][:, ds(qb * QB + sc, w)],
                        start=True,
                        stop=True,
                    )
                    if off >= 0:
                        # only the triangular block (plus, for off=3, the
                        # fully-invalid 128 cols kept for moving-dim>=256)
                        # needs masking; columns right of it are all-valid
                        msc = off * 128 if off < 3 else 256
                        mw = 128 if off < 3 else 256
                        nc.vector.tensor_add(
                            ps_sc[:, ds(msc, mw)],
                            ps_sc[:, ds(msc, mw)],
                            masks_sb[:, ds(off * QB + msc, mw)],
                        )
                    ex = epool.tile([128, QB], MMDT, tag="e", name="ex")
                    nc.scalar.activation(
                        ex[:, ds(sc, w)], ps_sc[:, ds(sc, w)], ACTF.Exp, scale=SCALE
                    )
                    nc.tensor.matmul(
                        ps_c[:, ds(sc, w)],
                        lhsT=v_sb[:, ts(kt, 128)],
                        rhs=ex[:, ds(sc, w)],
                        start=(kt == 0),
                        stop=(kt == n_kt - 1),
                        skip_group_check=True,
                    )
                    nc.tensor.matmul(
                        ps_l[:, ds(sc, w)],
                        lhsT=ones_sb[:],
                        rhs=ex[:, ds(sc, w)],
                        start=(kt == 0),
                        stop=(kt == n_kt - 1),
                        skip_group_check=True,
                    )
                rec = lpool.tile([1, QB], F32, tag="r", name="rec")
                nc.vector.reciprocal(rec[:], ps_l[:])
                bc = bcpool.tile([128, QB], F32, tag="bc", name="bc")
                nc.gpsimd.partition_broadcast(bc[:], rec[:])
                nc.vector.tensor_mul(
                    ctx_sbs[h][:, ts(qb, QB)], ps_c[:], bc[:]
                )

    # ---------------- Phase 3: output projection ----------------
    with (
        tc.tile_pool(name="o_stage", bufs=4) as ostage,
        tc.tile_pool(name="p3_psum", bufs=4, space="PSUM") as psum3,
    ):
        for m in range(n_sk):
            for n in range(D // QB):
                ps = psum3.tile([128, QB], F32, tag="o", name="ps_p3")
                for k in range(n_dg):
                    nc.tensor.matmul(
                        ps[:],
                        lhsT=ctx_sbs[k][:, ts(m, 128)],
                        rhs=wo_sb[:, ds(k * D + n * QB, QB)],
                        start=(k == 0),
                        stop=(k == n_dg - 1),
                    )
                ot = ostage.tile([128, QB], F32, tag="ot", name="ot")
                nc.vector.tensor_add(ot[:], ps[:], bo_sb[:, ts(n, QB)])
                nc.sync.dma_start(out[ts(m, 128), ts(n, QB)], ot[:])


def build_program(S=SEQ, D=D_MODEL, DGl=DG, enable_asserts=False):
    nc = bacc.Bacc(
        "TRN2",
        target_bir_lowering=False,
        debug=False,
        enable_asserts=enable_asserts,
        num_devices=N_CORES,
    )
    aps = {
        "xt": nc.dram_tensor("xt", [D, S], F32, kind="ExternalInput").ap(),
        "wqt": nc.dram_tensor("wqt", [D, DGl], F32, kind="ExternalInput").ap(),
        "wkt": nc.dram_tensor("wkt", [D, DGl], F32, kind="ExternalInput").ap(),
        "wvt": nc.dram_tensor("wvt", [D, DGl], F32, kind="ExternalInput").ap(),
        "wot": nc.dram_tensor("wot", [DGl, D], F32, kind="ExternalInput").ap(),
        "bq": nc.dram_tensor("bq", [128, DGl // 128], F32, kind="ExternalInput").ap(),
        "bk": nc.dram_tensor("bk", [128, DGl // 128], F32, kind="ExternalInput").ap(),
        "bv": nc.dram_tensor("bv", [128, DGl], F32, kind="ExternalInput").ap(),
        "bo": nc.dram_tensor("bo", [128, D], F32, kind="ExternalInput").ap(),
        "masks": nc.dram_tensor("masks", [4, 128, QB], F32, kind="ExternalInput").ap(),
        "ones": nc.dram_tensor("ones", [128, 1], F32, kind="ExternalInput").ap(),
        "out": nc.dram_tensor("out", [S, D], F32, kind="ExternalOutput").ap(),
    }
    with tile.TileContext(nc) as tc:
        with ExitStack() as ctx:
            _mha_body(ctx, tc, aps, S, D, DGl)
    nc.compile()
    return nc


def make_masks():
    """Additive causal masks: 0 where k<=q, -1e30 where masked."""
    i = np.arange(4)[:, None, None]
    p = np.arange(128)[None, :, None]
    f = np.arange(QB)[None, None, :]
    keep = (i * 128 + p) <= f
    return np.where(keep, 0.0, -1e30).astype(np.float32)


def shard_inputs(x, wq, bq, wk, bk, wv, bv, wo, bo):
    """Build the 8 per-core input maps (host-side layout prep)."""
    masks = make_masks()
    xts = [np.ascontiguousarray(np.asarray(x[b], np.float32).T) for b in range(BATCH)]
    bo_bc = np.ascontiguousarray(
        np.broadcast_to(np.asarray(bo, np.float32), (128, D_MODEL))
    )
    bo_zero = np.zeros((128, D_MODEL), np.float32)
    in_maps = []
    for c in range(N_CORES):
        b, g = divmod(c, N_GROUPS)
        sl = slice(g * DG, (g + 1) * DG)
        in_maps.append(
            {
                "xt": xts[b],
                "wqt": np.ascontiguousarray(np.asarray(wq, np.float32)[sl].T),
                "wkt": np.ascontiguousarray(np.asarray(wk, np.float32)[sl].T),
                "wvt": np.ascontiguousarray(np.asarray(wv, np.float32)[sl].T),
                "wot": np.ascontiguousarray(np.asarray(wo, np.float32)[:, sl].T),
                "bq": np.ascontiguousarray(
                    np.asarray(bq, np.float32)[sl].reshape(-1, 128).T
                ),
                "bk": np.ascontiguousarray(
                    np.asarray(bk, np.float32)[sl].reshape(-1, 128).T
                ),
                "bv": np.ascontiguousarray(
                    np.broadcast_to(np.asarray(bv, np.float32)[sl], (128, DG))
                ),
                "bo": bo_bc if g == 0 else bo_zero,
                "masks": masks,
                "ones": np.ones((128, 1), np.float32),
            }
        )
    return in_maps


_NC_CACHE = {}


def get_program():
    if "nc" not in _NC_CACHE:
        _NC_CACHE["nc"] = build_program()
    return _NC_CACHE["nc"]


def run_sharded(inputs, trace=False):
    nc = get_program()
    in_maps = shard_inputs(**inputs)
    res = run_bass_kernel_spmd(nc, in_maps, list(range(N_CORES)), trace=trace)
    full = np.empty((BATCH, SEQ, D_MODEL), np.float32)
    for b in range(BATCH):
        acc = res.results[b * N_GROUPS]["out"].copy()
        for g in range(1, N_GROUPS):
            acc += res.results[b * N_GROUPS + g]["out"]
        full[b] = acc
    return full, res


def kernel(**inputs):
    out, _ = run_sharded(inputs, trace=False)
    return out
